# revision 1
# baseline (speedup 1.0000x reference)
"""Trainium2 Bass kernel for nn_CausalNet (block-diagonal GNN + BN + MLP head).

Strategy: data-parallel over batch (8 samples/core on 8 cores).
 - Feature-major layouts throughout so every BN/broadcast is per-partition.
 - Both D^-1/2 scales folded into the stationary A^T via rank-1 outer matmuls.
 - BatchNorm stats cross-core via 2KB AllReduce x2.
 - Readout [64,131072]@[131072,128]: AllToAll redistributes h2 so each core
   contracts only its 16384-row slice of Wm1 (8.4MB HBM vs 67MB replicated),
   then a 32KB AllReduce combines partial products; tiny head replicated.
"""
import sys
import numpy as np

sys.path.insert(0, "/opt/trn_rl_repo")

B, N, P, D = 64, 4, 128, 256
H = 256
TOTP = N * P          # 512
NCORES = 8
BLOC = B // NCORES    # 8 samples per core
T = BLOC * TOTP       # 4096 tokens per core
NB = BLOC * N         # 32 (sample, subgraph) blocks per core
FEAT = TOTP * H       # 131072
FSL = FEAT // NCORES  # 16384 Wm1 rows per core
TSL = TOTP // NCORES  # 64 patches per core slice
JT = H // 128         # 2 feature partition-tiles
EPS_BN = 1e-5
CNT1 = float(B * TOTP)   # BN denominator for GCN layers
CNT2 = float(B)          # BN denominator for head


def build_bass(repeat=1, no_cc=False):
    import concourse.bass as bass
    import concourse.bacc as bacc
    import concourse.mybir as mybir
    import concourse.tile as tile

    f32 = mybir.dt.float32
    Act = mybir.ActivationFunctionType
    Alu = mybir.AluOpType
    AX = mybir.AxisListType

    nc = bacc.Bacc("TRN2", target_bir_lowering=False, debug=False,
                   num_devices=NCORES)

    def inp(name, shape):
        return nc.dram_tensor(name, shape, f32, kind="ExternalInput")

    xT_d = inp("xT", [D, T])            # d-major activations for this core
    W1_d = inp("W1", [D, H])
    b1_d = inp("b1r", [1, H])
    g1_d = inp("g1p", [128, JT])        # column jh = features [jh*128,(jh+1)*128)
    be1_d = inp("be1p", [128, JT])
    W2_d = inp("W2", [H, H])
    b2_d = inp("b2r", [1, H])
    g2_d = inp("g2p", [128, JT])
    be2_d = inp("be2p", [128, JT])
    mAT_d = inp("mAT", [P, 4 * P])      # (0.5*mask*(1-I)).T tiled x4
    mBT_d = inp("mBT", [P, 4 * P])      # (0.5*mask*(1-I) + I).T tiled x4
    Wm1_d = inp("Wm1s", [FSL, 128])     # this core's Wm1 slice
    gm1_d = inp("gm1", [128, 1])
    bem1_d = inp("bem1", [128, 1])
    Wm2_d = inp("Wm2", [128, 64])
    gm2_d = inp("gm2", [64, 1])
    bem2_d = inp("bem2", [64, 1])
    Wm3_d = inp("Wm3", [64, 2])
    bm3_d = inp("bm3", [2, 1])
    onec_d = inp("ones_col", [128, 1])
    oner_d = inp("ones_row", [1, 128])
    id_d = inp("ident", [128, 128])

    out_ext = nc.dram_tensor("out", [2, B], f32, kind="ExternalOutput")

    with tile.TileContext(nc) as tc:
        with (
            tc.tile_pool(name="persist", bufs=1) as pp,
            tc.tile_pool(name="work", bufs=3) as wp,
            tc.tile_pool(name="small", bufs=2) as sp,
            tc.tile_pool(name="scratch", bufs=1) as scp,
            tc.tile_pool(name="wm1", bufs=4) as wmp,
            tc.tile_pool(name="ps", bufs=2, space="PSUM") as ps,
            tc.tile_pool(name="dram", bufs=1, space="DRAM") as dp,
        ):
            # ---------------- persistent SBUF ----------------
            def load(name, dram, shape, sl=None):
                t = pp.tile(shape, f32, tag=name, name=name)
                nc.gpsimd.dma_start(out=t[:], in_=dram[:] if sl is None else sl)
                return t

            xTs = [load(f"xT{k}", xT_d, [128, T], xT_d[k * 128:(k + 1) * 128, :])
                   for k in range(2)]
            W1s = [load(f"W1{k}", W1_d, [128, H], W1_d[k * 128:(k + 1) * 128, :])
                   for k in range(2)]
            W2s = [load(f"W2{k}", W2_d, [128, H], W2_d[k * 128:(k + 1) * 128, :])
                   for k in range(2)]
            b1s = load("b1", b1_d, [1, H])
            b2s = load("b2", b2_d, [1, H])
            mATs = load("mAT", mAT_d, [P, 4 * P])
            mBTs = load("mBT", mBT_d, [P, 4 * P])
            onec = load("onec", onec_d, [128, 1])
            oner = load("oner", oner_d, [1, 128])
            idents = load("ident", id_d, [128, 128])
            g1s = load("g1", g1_d, [128, JT])
            be1s = load("be1", be1_d, [128, JT])
            g2s = load("g2", g2_d, [128, JT])
            be2s = load("be2", be2_d, [128, JT])
            gm1s = load("gm1", gm1_d, [128, 1])
            bem1s = load("bem1", bem1_d, [128, 1])
            gm2s = load("gm2", gm2_d, [64, 1])
            bem2s = load("bem2", bem2_d, [64, 1])
            Wm2s = load("Wm2", Wm2_d, [128, 64])
            Wm3s = load("Wm3", Wm3_d, [64, 2])
            bm3s = load("bm3", bm3_d, [2, 1])

            epsb = pp.tile([128, 1], f32, tag="epsb")
            nc.vector.memset(epsb[:], EPS_BN)
            rinv = pp.tile([1, T], f32, tag="rinv")          # 1/||x_t||
            AnT = pp.tile([128, NB * P], f32, tag="AnT")     # scaled A^T blocks
            h1T = [pp.tile([128, T], f32, tag=f"h1T{k}", name=f"h1T{k}") for k in range(JT)]
            h2T = [pp.tile([128, T], f32, tag=f"h2T{k}", name=f"h2T{k}") for k in range(JT)]

            rg = [list(range(NCORES))]

            def cc(kind, op, cin, cout):
                if no_cc:
                    nc.sync.dma_start(out=cout[:], in_=cin[:])
                else:
                    nc.gpsimd.collective_compute(
                        kind, op, replica_groups=rg,
                        ins=[cin.opt()], outs=[cout.opt()])

            for _rep in range(repeat):
                st1_in = dp.tile([128, 4], f32, tag="st1i", name="st1_in")
                st1_out = dp.tile([128, 4], f32, tag="st1o", addr_space="Shared", name="st1_out")
                st2_in = dp.tile([128, 4], f32, tag="st2i", name="st2_in")
                st2_out = dp.tile([128, 4], f32, tag="st2o", addr_space="Shared", name="st2_out")
                a2a_in = dp.tile([NCORES, BLOC, H, TSL], f32, tag="a2ai", name="a2a_in")
                a2a_out = dp.tile([NCORES, BLOC, H, TSL], f32, tag="a2ao", name="a2a_out")
                z1_in = dp.tile([64, 128], f32, tag="z1i", name="z1_in")
                z1_out = dp.tile([64, 128], f32, tag="z1o", addr_space="Shared", name="z1_out")
                # ---------------- row norms: rinv[t] = 1/||x_t|| ----------------
                sq = scp.tile([128, T], f32, tag="sq")
                nc.scalar.activation(sq[:], xTs[0][:], Act.Square)
                nc.gpsimd.tensor_reduce(rinv[:], sq[:], AX.C, op=Alu.add)
                sq = scp.tile([128, T], f32, tag="sq")
                nc.scalar.activation(sq[:], xTs[1][:], Act.Square)
                red1 = sp.tile([1, T], f32, tag="red1", bufs=1)
                nc.gpsimd.tensor_reduce(red1[:], sq[:], AX.C, op=Alu.add)
                nc.vector.tensor_add(rinv[:], rinv[:], red1[:])
                nc.scalar.activation(rinv[:], rinv[:], Act.Sqrt)
                nc.vector.reciprocal(rinv[:], rinv[:])

                # ------- adjacency + layer1, 4 blocks per PSUM bank -------
                SB = NB // 4
                for sb in range(SB):
                    c0 = sb * 4 * P
                    G4 = ps.tile([P, 4 * P], f32, tag="G")
                    for b in range(4):
                        cb = c0 + b * P
                        for kt in range(2):
                            nc.tensor.matmul(
                                G4[:, b * P:(b + 1) * P],
                                xTs[kt][:, cb:cb + P], xTs[kt][:, cb:cb + P],
                                start=(kt == 0), stop=(kt == 1),
                            )
                    R4 = ps.tile([P, 4 * P], f32, tag="adj")
                    for b in range(4):
                        cb = c0 + b * P
                        nc.tensor.matmul(R4[:, b * P:(b + 1) * P],
                                         rinv[:, cb:cb + P], rinv[:, cb:cb + P],
                                         start=True, stop=True)
                    t1 = wp.tile([P, 4 * P], f32, tag="t1")
                    nc.vector.tensor_mul(t1[:], G4[:], mATs[:])
                    AT = wp.tile([P, 4 * P], f32, tag="AT")
                    nc.vector.tensor_mul(AT[:], R4[:], t1[:])
                    nc.vector.tensor_add(AT[:], AT[:], mBTs[:])
                    dg = sp.tile([1, 4 * P], f32, tag="dg", bufs=1)
                    nc.gpsimd.tensor_reduce(dg[:], AT[:], AX.C, op=Alu.add)
                    dr = sp.tile([1, 4 * P], f32, tag="dr", bufs=1)
                    nc.scalar.activation(dr[:], dg[:], Act.Sqrt)
                    dinv = sp.tile([1, 4 * P], f32, tag="dinv")
                    nc.vector.reciprocal(dinv[:], dr[:])
                    Do4 = ps.tile([P, 4 * P], f32, tag="adj")
                    for b in range(4):
                        nc.tensor.matmul(Do4[:, b * P:(b + 1) * P],
                                         dinv[:, b * P:(b + 1) * P],
                                         dinv[:, b * P:(b + 1) * P],
                                         start=True, stop=True)
                    nc.vector.tensor_mul(AnT[:, c0:c0 + 4 * P], AT[:], Do4[:])

                    xws = []
                    for b in range(4):
                        cb = c0 + b * P
                        xw_ps = ps.tile([128, H], f32, tag="xw")
                        for kt in range(2):
                            nc.tensor.matmul(
                                xw_ps[:], xTs[kt][:, cb:cb + P], W1s[kt][:],
                                start=(kt == 0), stop=False,
                            )
                        nc.tensor.matmul(xw_ps[:], oner[:], b1s[:],
                                         start=False, stop=True)
                        xw = wp.tile([128, H], f32, tag="xw", bufs=5)
                        nc.vector.tensor_copy(xw[:], xw_ps[:])
                        xws.append(xw)

                    for jh in range(JT):
                        hh4 = ps.tile([128, 4 * P], f32, tag="hh")
                        for b in range(4):
                            cb = c0 + b * P
                            nc.tensor.matmul(
                                hh4[:, b * P:(b + 1) * P],
                                xws[b][:, jh * 128:(jh + 1) * 128],
                                AnT[:, cb:cb + P],
                                start=True, stop=True,
                            )
                        nc.vector.tensor_copy(h1T[jh][:, c0:c0 + 4 * P], hh4[:])

                # ---------------- BN stats + allreduce + apply (shared) --------
                def bn_gcn(hT, stin, stout, gs, bes):
                    st = sp.tile([128, 4], f32, tag="st")
                    for jh in range(JT):
                        nc.vector.reduce_sum(st[:, jh:jh + 1], hT[jh][:], AX.X)
                        sq = scp.tile([128, T], f32, tag="sq")
                        nc.scalar.activation(sq[:], hT[jh][:], Act.Square,
                                             accum_out=st[:, 2 + jh:3 + jh])
                    nc.sync.dma_start(out=stin[:], in_=st[:])
                    nc.gpsimd.collective_compute(
                        "AllReduce", Alu.add, replica_groups=rg,
                        ins=[stin.opt()], outs=[stout.opt()],
                    )
                    stg = sp.tile([128, 4], f32, tag="stg")
                    nc.sync.dma_start(out=stg[:], in_=stout[:])
                    for jh in range(JT):
                        mean = sp.tile([128, 1], f32, tag="mean")
                        nc.vector.tensor_scalar_mul(mean[:], stg[:, jh:jh + 1], 1.0 / CNT1)
                        msq = sp.tile([128, 1], f32, tag="msq")
                        nc.vector.tensor_mul(msq[:], mean[:], mean[:])
                        var = sp.tile([128, 1], f32, tag="var")
                        nc.vector.tensor_scalar_mul(var[:], stg[:, 2 + jh:3 + jh],
                                                    1.0 / CNT1)
                        nc.vector.tensor_sub(var[:], var[:], msq[:])
                        sd = sp.tile([128, 1], f32, tag="sd")
                        nc.scalar.activation(sd[:], var[:], Act.Sqrt, bias=epsb[:var.shape[0], :])
                        rsd = sp.tile([128, 1], f32, tag="rsd")
                        nc.vector.reciprocal(rsd[:], sd[:])
                        a = sp.tile([128, 1], f32, tag="a")
                        nc.vector.tensor_mul(a[:], gs[:, jh:jh + 1], rsd[:])
                        c = sp.tile([128, 1], f32, tag="c")
                        nc.vector.tensor_mul(c[:], mean[:], a[:])
                        nc.vector.tensor_sub(c[:], bes[:, jh:jh + 1], c[:])
                        nc.scalar.activation(hT[jh][:], hT[jh][:], Act.Relu,
                                             bias=c[:], scale=a[:])

                bn_gcn(h1T, st1_in, st1_out, g1s, be1s)

                # ---------------- layer 2 (4-block batches) ----------------
                for sb in range(SB):
                    c0 = sb * 4 * P
                    xws = []
                    for b in range(4):
                        cb = c0 + b * P
                        xw_ps = ps.tile([128, H], f32, tag="xw")
                        for jh in range(JT):
                            nc.tensor.matmul(
                                xw_ps[:], h1T[jh][:, cb:cb + P], W2s[jh][:],
                                start=(jh == 0), stop=False,
                            )
                        nc.tensor.matmul(xw_ps[:], oner[:], b2s[:],
                                         start=False, stop=True)
                        xw = wp.tile([128, H], f32, tag="xw", bufs=5)
                        nc.vector.tensor_copy(xw[:], xw_ps[:])
                        xws.append(xw)
                    for jh in range(JT):
                        hh4 = ps.tile([128, 4 * P], f32, tag="hh")
                        for b in range(4):
                            cb = c0 + b * P
                            nc.tensor.matmul(
                                hh4[:, b * P:(b + 1) * P],
                                xws[b][:, jh * 128:(jh + 1) * 128],
                                AnT[:, cb:cb + P],
                                start=True, stop=True,
                            )
                        nc.vector.tensor_copy(h2T[jh][:, c0:c0 + 4 * P], hh4[:])

                bn_gcn(h2T, st2_in, st2_out, g2s, be2s)

                # ---------------- AllToAll export of h2 ----------------
                # a2a_in[c', s, j, t64] = h2T[j][s*512 + c'*64 + t64]
                for cd in range(NCORES):
                    for s in range(BLOC):
                        for jh in range(JT):
                            dst = a2a_in[cd, s, jh * 128:(jh + 1) * 128, :]
                            src = h2T[jh][:, s * TOTP + cd * TSL:
                                          s * TOTP + cd * TSL + TSL]
                            nc.sync.dma_start(out=dst, in_=src)
                cc("AllToAll", Alu.bypass, a2a_in, a2a_out)

                # ---------------- readout partial z1 ----------------
                z1p = ps.tile([64, 128], f32, tag="G")
                NK = FSL // 128
                a2a_flat = a2a_out.rearrange("r s j t -> r s (j t)")
                for k in range(NK):
                    lhs = wp.tile([128, 64], f32, tag="lhs")
                    src = a2a_flat[:, :, k * 128:(k + 1) * 128]  # [8, 8, 128]
                    src = src.rearrange("r s f -> f r s")
                    nc.sync.dma_start(out=lhs[:].rearrange("f (r s) -> f r s", r=NCORES),
                                      in_=src)
                    wk = wmp.tile([128, 128], f32, tag="wk")
                    nc.sync.dma_start(out=wk[:], in_=Wm1_d[k * 128:(k + 1) * 128, :])
                    nc.tensor.matmul(z1p[:], lhs[:], wk[:],
                                     start=(k == 0), stop=(k == NK - 1))
                z1s = sp.tile([64, 128], f32, tag="z1s")
                nc.vector.tensor_copy(z1s[:], z1p[:])
                nc.sync.dma_start(out=z1_in[:], in_=z1s[:])
                cc("AllReduce", Alu.add, z1_in, z1_out)
                z1g = sp.tile([64, 128], f32, tag="z1g")
                nc.sync.dma_start(out=z1g[:], in_=z1_out[:])

                z1t_ps = ps.tile([128, 64], f32, tag="hh")
                nc.tensor.transpose(z1t_ps[:], z1g[:], idents[:64, :64])
                z1t = sp.tile([128, 64], f32, tag="z1t")
                nc.vector.tensor_copy(z1t[:], z1t_ps[:])

                # ---------------- head BN + relu ----------------
                def head_bn(zt, parts, gs, bes):
                    stm = sp.tile([parts, 1], f32, tag="hstm")
                    nc.vector.reduce_sum(stm[:], zt[:], AX.X)
                    mean = sp.tile([parts, 1], f32, tag="hmean")
                    nc.vector.tensor_scalar_mul(mean[:], stm[:], 1.0 / CNT2)
                    sqs2 = sp.tile([parts, 64], f32, tag="hsq")
                    sts = sp.tile([parts, 1], f32, tag="hsts")
                    nc.scalar.activation(sqs2[:], zt[:], Act.Square, accum_out=sts[:])
                    var = sp.tile([parts, 1], f32, tag="hvar")
                    nc.vector.tensor_scalar_mul(var[:], sts[:], 1.0 / CNT2)
                    msq = sp.tile([parts, 1], f32, tag="hmsq")
                    nc.vector.tensor_mul(msq[:], mean[:], mean[:])
                    nc.vector.tensor_sub(var[:], var[:], msq[:])
                    sd = sp.tile([parts, 1], f32, tag="hsd")
                    nc.scalar.activation(sd[:], var[:], Act.Sqrt, bias=epsb[:var.shape[0], :])
                    rsd = sp.tile([parts, 1], f32, tag="hrsd")
                    nc.vector.reciprocal(rsd[:], sd[:])
                    a = sp.tile([parts, 1], f32, tag="ha")
                    nc.vector.tensor_mul(a[:], gs[:], rsd[:])
                    c = sp.tile([parts, 1], f32, tag="hc")
                    nc.vector.tensor_mul(c[:], mean[:], a[:])
                    nc.vector.tensor_sub(c[:], bes[:], c[:])
                    nc.scalar.activation(zt[:], zt[:], Act.Relu, bias=c[:], scale=a[:])

                head_bn(z1t, 128, gm1s, bem1s)

                z2_ps = ps.tile([64, 64], f32, tag="adj")
                nc.tensor.matmul(z2_ps[:], Wm2s[:], z1t[:], start=True, stop=True)
                z2t = sp.tile([64, 64], f32, tag="z2t")
                nc.vector.tensor_copy(z2t[:], z2_ps[:])
                head_bn(z2t, 64, gm2s, bem2s)

                z3_ps = ps.tile([2, 64], f32, tag="adj")
                nc.tensor.matmul(z3_ps[:], Wm3s[:], z2t[:], start=True, stop=True)
                z3 = sp.tile([2, 64], f32, tag="z3")
                nc.vector.tensor_scalar_add(z3[:], z3_ps[:], bm3s[:])
                nc.sync.dma_start(out=out_ext[:], in_=z3[:])

    nc.finalize()
    return nc


_CACHE = {}


def prepare_in_maps(inputs):
    x = np.asarray(inputs["x"], np.float32)
    mask = np.asarray(inputs["edge_prior_mask"], np.float32)
    Wm1 = np.asarray(inputs["Wm1"], np.float32)

    mA = 0.5 * mask * (1.0 - np.eye(P, dtype=np.float32))
    mB = mA + np.eye(P, dtype=np.float32)

    def c2(v, parts):  # [2*parts] -> [parts, 2] column-per-tile packing
        return np.ascontiguousarray(
            np.asarray(v, np.float32).reshape(2, parts).T)

    common = {
        "W1": np.asarray(inputs["W1"], np.float32),
        "b1r": np.asarray(inputs["b1"], np.float32).reshape(1, H),
        "g1p": c2(inputs["g1"], 128), "be1p": c2(inputs["be1"], 128),
        "W2": np.asarray(inputs["W2"], np.float32),
        "b2r": np.asarray(inputs["b2"], np.float32).reshape(1, H),
        "g2p": c2(inputs["g2"], 128), "be2p": c2(inputs["be2"], 128),
        "mAT": np.ascontiguousarray(np.tile(mA.T, (1, 4))),
        "mBT": np.ascontiguousarray(np.tile(mB.T, (1, 4))),
        "gm1": np.asarray(inputs["gm1"], np.float32).reshape(128, 1),
        "bem1": np.asarray(inputs["bem1"], np.float32).reshape(128, 1),
        "Wm2": np.asarray(inputs["Wm2"], np.float32),
        "gm2": np.asarray(inputs["gm2"], np.float32).reshape(64, 1),
        "bem2": np.asarray(inputs["bem2"], np.float32).reshape(64, 1),
        "Wm3": np.asarray(inputs["Wm3"], np.float32),
        "bm3": np.asarray(inputs["bm3"], np.float32).reshape(2, 1),
        "ones_col": np.ones((128, 1), np.float32),
        "ones_row": np.ones((1, 128), np.float32),
        "ident": np.eye(128, dtype=np.float32),
    }
    in_maps = []
    for c in range(NCORES):
        xc = x[c * BLOC:(c + 1) * BLOC].reshape(T, D)
        m = dict(common)
        m["xT"] = np.ascontiguousarray(xc.T)
        ws = Wm1[c * FSL:(c + 1) * FSL, :].reshape(TSL, H, 128)
        m["Wm1s"] = np.ascontiguousarray(
            ws.transpose(1, 0, 2).reshape(FSL, 128))
        in_maps.append(m)
    return in_maps


def kernel(**inputs):
    import concourse.bass_utils as bass_utils

    in_maps = prepare_in_maps(inputs)
    if "nc" not in _CACHE:
        _CACHE["nc"] = build_bass()
    res = bass_utils.run_bass_kernel_spmd(
        _CACHE["nc"], in_maps, core_ids=list(range(NCORES)))
    _CACHE["last"] = res
    out = res.results[0]["out"]  # [2, 64]
    return np.ascontiguousarray(np.asarray(out).T)



# revision 13
# speedup vs baseline: 9.4490x; 9.4490x over previous
"""Trainium2 Bass kernel for nn_CausalNet (block-diagonal GNN + BN + MLP head).

Data-parallel over batch (8 samples/core on 8 cores). v2 design:
 - bf16 matmuls everywhere in the GNN body (fp32 PSUM accumulation).
 - No gpsimd partition reduces: row norms and degrees via ones-vector
   matmuls (partition contraction on the PE), rsqrt via Sqrt +
   reciprocal_approx_fast.
 - Wm1 (bf16, 4.2MB/core) SBUF-resident via one early DMA that overlaps
   the whole GCN phase; readout matmuls read SBUF directly.
 - AllToAll payload is pre-BN h2 in bf16 with 1KB-contiguous lines both
   directions; BN2 stats AllReduce rides concurrently and BN2+relu is
   applied post-exchange on the received tiles.
 - BN statistics fused into PSUM evacuation (activation accum_out).
"""
import sys
import numpy as np

sys.path.insert(0, "/opt/trn_rl_repo")

B, N, P, D = 64, 4, 128, 256
H = 256
TOTP = N * P          # 512
NCORES = 8
BLOC = B // NCORES    # 8 samples per core
T = BLOC * TOTP       # 4096 tokens per core
NB = BLOC * N         # 32 (sample, subgraph) blocks per core
SB = NB // 4          # 8 groups of 4 blocks
FEAT = TOTP * H       # 131072
FSL = FEAT // NCORES  # 16384 Wm1 rows per core
TSL = TOTP // NCORES  # 64 patches per a2a slice
JT = H // 128         # 2 feature partition-tiles
NTAU = JT * TSL       # 128 readout k-tiles
EPS_BN = 1e-5
CNT1 = float(B * TOTP)   # BN denominator for GCN layers
CNT2 = float(B)          # BN denominator for head


def build_bass(no_cc=False):
    import concourse.bass as bass
    import concourse.bacc as bacc
    import concourse.mybir as mybir
    import concourse.tile as tile

    f32 = mybir.dt.float32
    f32r = mybir.dt.float32r
    bf16 = mybir.dt.bfloat16
    Act = mybir.ActivationFunctionType
    Alu = mybir.AluOpType
    AX = mybir.AxisListType

    nc = bacc.Bacc("TRN2", target_bir_lowering=False, debug=False,
                   num_devices=NCORES)

    def inp(name, shape, dt=f32):
        return nc.dram_tensor(name, shape, dt, kind="ExternalInput")

    xT_d = inp("xT", [D, T], bf16)       # d-major activations, bf16
    W1_d = inp("W1", [D, H], bf16)
    b1_d = inp("b1r", [1, H], bf16)
    W2_d = inp("W2", [H, H], bf16)
    b2_d = inp("b2r", [1, H], bf16)
    g1_d = inp("g1p", [128, JT])
    be1_d = inp("be1p", [128, JT])
    g2_d = inp("g2p", [128, JT])
    be2_d = inp("be2p", [128, JT])
    mAT_d = inp("mAT", [P, 4 * P])       # (0.5*mask*(1-I)).T tiled x4
    mBT_d = inp("mBT", [P, 4 * P])       # mAT + I tiled x4
    Wm1_d = inp("Wm1s", [128, NTAU * 128], bf16)  # [f_lane, (jh,pt,o)]
    gm1_d = inp("gm1", [128, 1])
    bem1_d = inp("bem1", [128, 1])
    Wm2_d = inp("Wm2", [128, 64])
    gm2_d = inp("gm2", [64, 1])
    bem2_d = inp("bem2", [64, 1])
    Wm3_d = inp("Wm3", [64, 2])
    bm3_d = inp("bm3", [2, 1])
    onerb_d = inp("ones_row16", [1, 128], bf16)
    onec_d = inp("ones_col", [128, 1])
    onecb_d = inp("ones_col16", [128, 1], bf16)
    id_d = inp("ident", [128, 128])

    out_ext = nc.dram_tensor("out", [2, B], f32, kind="ExternalOutput")

    with tile.TileContext(nc) as tc:
        with (
            tc.tile_pool(name="persist", bufs=1) as pp,
            tc.tile_pool(name="xsq", bufs=1) as xqp,
            tc.tile_pool(name="work", bufs=3) as wp,
            tc.tile_pool(name="xw", bufs=6) as xwp,
            tc.tile_pool(name="small", bufs=2) as sp,
            tc.tile_pool(name="rows", bufs=1) as rp,
            tc.tile_pool(name="ps", bufs=2, space="PSUM") as ps,
            tc.tile_pool(name="dram", bufs=1, space="DRAM") as dp,
        ):
            # ---------------- persistent SBUF / initial DMAs ----------------
            def load(name, dram, shape, dt=f32, sl=None, eng=None):
                t = pp.tile(shape, dt, tag=name, name=name)
                e = eng or nc.sync
                e.dma_start(out=t[:], in_=dram[:] if sl is None else sl)
                return t

            # Wm1 first: 4.2MB, overlaps the whole GCN phase (gpsimd queue).
            Wm1s = load("Wm1s", Wm1_d, [128, NTAU * 128], bf16, eng=nc.gpsimd)
            xbf = [load(f"xT{k}", xT_d, [128, T], bf16,
                        xT_d[k * 128:(k + 1) * 128, :])
                   for k in range(2)]
            W1s = [load(f"W1{k}", W1_d, [128, H], bf16,
                        W1_d[k * 128:(k + 1) * 128, :], eng=nc.scalar)
                   for k in range(2)]
            W2s = [load(f"W2{k}", W2_d, [128, H], bf16,
                        W2_d[k * 128:(k + 1) * 128, :], eng=nc.scalar)
                   for k in range(2)]
            b1s = load("b1", b1_d, [1, H], bf16, eng=nc.scalar)
            b2s = load("b2", b2_d, [1, H], bf16, eng=nc.scalar)
            mATs = load("mAT", mAT_d, [P, 4 * P], eng=nc.scalar)
            mBTs = load("mBT", mBT_d, [P, 4 * P], eng=nc.scalar)
            onerb = load("onerb", onerb_d, [1, 128], bf16, eng=nc.scalar)
            onec = load("onec", onec_d, [128, 1], eng=nc.scalar)
            onecb = load("onecb", onecb_d, [128, 1], bf16, eng=nc.scalar)
            idents = load("ident", id_d, [128, 128], eng=nc.scalar)
            g1s = load("g1", g1_d, [128, JT], eng=nc.scalar)
            be1s = load("be1", be1_d, [128, JT], eng=nc.scalar)
            g2s = load("g2", g2_d, [128, JT], eng=nc.scalar)
            be2s = load("be2", be2_d, [128, JT], eng=nc.scalar)
            gm1s = load("gm1", gm1_d, [128, 1], eng=nc.scalar)
            bem1s = load("bem1", bem1_d, [128, 1], eng=nc.scalar)
            gm2s = load("gm2", gm2_d, [64, 1], eng=nc.scalar)
            bem2s = load("bem2", bem2_d, [64, 1], eng=nc.scalar)
            Wm2s = load("Wm2", Wm2_d, [128, 64], eng=nc.scalar)
            Wm3s = load("Wm3", Wm3_d, [64, 2], eng=nc.scalar)
            bm3s = load("bm3", bm3_d, [2, 1], eng=nc.scalar)

            epsb = pp.tile([128, 1], f32, tag="epsb")
            nc.vector.memset(epsb[:], EPS_BN)

            rinv = pp.tile([1, T], bf16, tag="rinv")     # 1/||x_t|| rows
            dinv = pp.tile([1, T], bf16, tag="dinv")     # deg^-1/2 rows

            AT_all = pp.tile([128, T], bf16, tag="ATall")  # A^T blocks
            AnT = pp.tile([128, T], bf16, tag="AnT")       # normalized A^T
            h1T = pp.tile([128, 2 * T], bf16, tag="h1T")   # [f, (jh, t)]
            h2T = pp.tile([128, 2 * T], bf16, tag="h2T")
            recv = [pp.tile([128, T], bf16, tag=f"recv{j}", name=f"recv{j}")
                    for j in range(JT)]
            sumb1 = pp.tile([128, 16], f32, tag="sumb1")   # per-group sums
            sumb2 = pp.tile([128, 16], f32, tag="sumb2")

            rg = [list(range(NCORES))]

            def cc(kind, op, cin, cout):
                if no_cc:
                    nc.sync.dma_start(out=cout[:], in_=cin[:])
                else:
                    nc.gpsimd.collective_compute(
                        kind, op, replica_groups=rg,
                        ins=[cin.opt()], outs=[cout.opt()])

            st1_in = dp.tile([128, 4], f32, tag="st1i", name="st1_in")
            st1_out = dp.tile([128, 4], f32, tag="st1o", addr_space="Shared",
                              name="st1_out")
            st2_in = dp.tile([128, 4], f32, tag="st2i", name="st2_in")
            st2_out = dp.tile([128, 4], f32, tag="st2o", addr_space="Shared",
                              name="st2_out")
            # a2a payload: [dest/src, jh, f, s, pt] — per (slot, jh, f) the
            # (s, pt) run is 1KB-contiguous both directions.
            a2a_in = dp.tile([NCORES, JT, 128, BLOC, TSL], bf16, tag="a2ai",
                             name="a2a_in")
            a2a_out = dp.tile([NCORES, JT, 128, BLOC, TSL], bf16, tag="a2ao",
                              name="a2a_out")
            z1_in = dp.tile([64, 128], f32, tag="z1i", name="z1_in")
            z1_out = dp.tile([64, 128], f32, tag="z1o", addr_space="Shared",
                             name="z1_out")

            # ---------------- row norms: rinv[t] = 1/||x_t|| ----------------
            rn_sb = rp.tile([1, T], f32, tag="rowf")
            xsq = xqp.tile([128, 2 * T], bf16, tag="xsq")
            for kt in range(2):
                nc.scalar.activation(xsq[:, kt * T:(kt + 1) * T], xbf[kt][:],
                                     Act.Square)
            for j in range(SB):
                ns = ps.tile([1, 512], f32, tag="rowmm")
                for kt in range(2):
                    nc.tensor.matmul(
                        ns[:], onecb[:],
                        xsq[:, kt * T + j * 512: kt * T + (j + 1) * 512],
                        start=(kt == 0), stop=(kt == 1))
                nc.scalar.activation(rn_sb[:, j * 512:(j + 1) * 512], ns[:],
                                     Act.Sqrt)
            nc.vector.reciprocal_approx_fast(rn_sb[:], rn_sb[:])
            nc.scalar.activation(rinv[:], rn_sb[:], Act.Copy)

            # ------- pass A: Gram, A^T blocks, degrees (4 blocks/group) -----
            dg_sb = rp.tile([1, T], f32, tag="rowf")
            for g in range(SB):
                c0 = g * 4 * P
                G4 = ps.tile([P, 4 * P], f32, tag="big", bufs=4)
                for b in range(4):
                    cb = c0 + b * P
                    for kt in range(2):
                        nc.tensor.matmul(
                            G4[:, b * P:(b + 1) * P],
                            xbf[kt][:, cb:cb + P], xbf[kt][:, cb:cb + P],
                            start=(kt == 0), stop=(kt == 1))
                R4 = ps.tile([P, 4 * P], f32, tag="big", bufs=4)
                for b in range(4):
                    cb = c0 + b * P
                    nc.tensor.matmul(R4[:, b * P:(b + 1) * P],
                                     rinv[:, cb:cb + P], rinv[:, cb:cb + P],
                                     start=True, stop=True)
                tt = wp.tile([P, 4 * P], f32, tag="tt")
                nc.vector.tensor_mul(tt[:], G4[:], mATs[:])
                nc.vector.tensor_mul(tt[:], R4[:], tt[:])
                nc.vector.tensor_add(AT_all[:, c0:c0 + 4 * P], tt[:], mBTs[:])
                dg = ps.tile([1, 512], f32, tag="rowmm")
                nc.tensor.matmul(dg[:], onecb[:],
                                 AT_all[:, c0:c0 + 4 * P],
                                 start=True, stop=True)
                nc.scalar.activation(dg_sb[:, c0:c0 + 4 * P], dg[:], Act.Sqrt)
            nc.vector.reciprocal_approx_fast(dg_sb[:], dg_sb[:])
            nc.scalar.activation(dinv[:], dg_sb[:], Act.Copy)

            # ------- pass B: An^T + layer 1 (4 blocks per group) ------------
            def xw_block(cb, src_fn, Ws, bs):
                """[128 q, 256 f] = x_block.T @ W + 1 b   (bf16)."""
                xw_ps = ps.tile([128, H], f32, tag="xwps")
                for kt in range(2):
                    nc.tensor.matmul(
                        xw_ps[:], src_fn(kt, cb),
                        Ws[kt][:], start=(kt == 0), stop=False)
                nc.tensor.matmul(xw_ps[:], onerb[:], bs[:],
                                 start=False, stop=True)
                xw = xwp.tile([128, H], bf16, tag="xw")
                nc.vector.tensor_copy(xw[:], xw_ps[:])
                return xw

            def layer(src_fn, Ws, bs, hT, sumb, build_an):
                for g in range(SB):
                    c0 = g * 4 * P
                    if build_an:
                        Do4 = ps.tile([P, 4 * P], f32, tag="big", bufs=4)
                        for b in range(4):
                            cb = c0 + b * P
                            nc.tensor.matmul(Do4[:, b * P:(b + 1) * P],
                                             dinv[:, cb:cb + P],
                                             dinv[:, cb:cb + P],
                                             start=True, stop=True)
                        nc.vector.tensor_mul(AnT[:, c0:c0 + 4 * P],
                                             AT_all[:, c0:c0 + 4 * P], Do4[:])
                    xws = [xw_block(c0 + b * P, src_fn, Ws, bs)
                           for b in range(4)]
                    for jh in range(JT):
                        hh4 = ps.tile([128, 4 * P], f32, tag="big", bufs=4)
                        for b in range(4):
                            cb = c0 + b * P
                            nc.tensor.matmul(
                                hh4[:, b * P:(b + 1) * P],
                                xws[b][:, jh * 128:(jh + 1) * 128],
                                AnT[:, cb:cb + P],
                                start=True, stop=True)
                        nc.scalar.activation(
                            hT[:, jh * T + c0: jh * T + c0 + 4 * P], hh4[:],
                            Act.Identity,
                            accum_out=sumb[:, g * 2 + jh: g * 2 + jh + 1])

            layer(lambda kt, cb: xbf[kt][:, cb:cb + P], W1s, b1s,
                  h1T, sumb1, build_an=True)

            # ---------------- BN stats + allreduce + apply ------------------
            def bn_stats(hT, sumb, stin, stout):
                st = sp.tile([128, 4], f32, tag="st")
                for jh in range(JT):
                    nc.vector.reduce_sum(
                        st[:, jh:jh + 1],
                        sumb[:].rearrange("f (g j) -> f j g", j=2)[:, jh, :],
                        AX.X)
                    sq = xqp.tile([128, 2 * T], bf16, tag="xsq")
                    nc.scalar.activation(sq[:, :T], hT[:, jh * T:(jh + 1) * T],
                                         Act.Square,
                                         accum_out=st[:, 2 + jh:3 + jh])
                nc.sync.dma_start(out=stin[:], in_=st[:])
                cc("AllReduce", Alu.add, stin, stout)
                stg = sp.tile([128, 4], f32, tag="stg")
                nc.sync.dma_start(out=stg[:], in_=stout[:])
                return stg

            def bn_coeffs(stg, jh, gs, bes):
                mean = sp.tile([128, 1], f32, tag="mean")
                nc.vector.tensor_scalar_mul(mean[:], stg[:, jh:jh + 1],
                                            1.0 / CNT1)
                msq = sp.tile([128, 1], f32, tag="msq")
                nc.vector.tensor_mul(msq[:], mean[:], mean[:])
                var = sp.tile([128, 1], f32, tag="var")
                nc.vector.tensor_scalar_mul(var[:], stg[:, 2 + jh:3 + jh],
                                            1.0 / CNT1)
                nc.vector.tensor_sub(var[:], var[:], msq[:])
                sd = sp.tile([128, 1], f32, tag="sd")
                nc.scalar.activation(sd[:], var[:], Act.Sqrt, bias=epsb[:])
                rsd = sp.tile([128, 1], f32, tag="rsd")
                nc.vector.reciprocal(rsd[:], sd[:])
                a = sp.tile([128, 1], f32, tag="a")
                nc.vector.tensor_mul(a[:], gs[:, jh:jh + 1], rsd[:])
                c = sp.tile([128, 1], f32, tag="c")
                nc.vector.tensor_mul(c[:], mean[:], a[:])
                nc.vector.tensor_sub(c[:], bes[:, jh:jh + 1], c[:])
                return a, c

            stg1 = bn_stats(h1T, sumb1, st1_in, st1_out)
            for jh in range(JT):
                a, c = bn_coeffs(stg1, jh, g1s, be1s)
                nc.scalar.activation(h1T[:, jh * T:(jh + 1) * T],
                                     h1T[:, jh * T:(jh + 1) * T],
                                     Act.Relu, bias=c[:], scale=a[:])

            # ---------------- layer 2 ----------------
            layer(lambda kt, cb: h1T[:, kt * T + cb: kt * T + cb + P],
                  W2s, b2s, h2T, sumb2, build_an=False)

            # ------- BN2 stats AR + AllToAll of pre-BN h2 (concurrent) ------
            st2g = bn_stats(h2T, sumb2, st2_in, st2_out)

            hv = h2T[:].rearrange("f (jh s dd p) -> f jh s dd p",
                                  jh=JT, s=BLOC, dd=NCORES, p=TSL)
            qs = [nc.sync, nc.scalar, nc.gpsimd]
            for dd in range(NCORES):
                for jh in range(JT):
                    qs[(dd * JT + jh) % 3].dma_start(
                        out=a2a_in[dd, jh], in_=hv[:, jh, :, dd, :])
            cc("AllToAll", Alu.bypass, a2a_in, a2a_out)

            for src in range(NCORES):
                for jh in range(JT):
                    qs[(src * JT + jh) % 3].dma_start(
                        out=recv[jh][:, src * 512:(src + 1) * 512]
                        .rearrange("f (s p) -> f s p", s=BLOC),
                        in_=a2a_out[src, jh])

            # BN2 + relu applied on the received (redistributed) tiles.
            for jh in range(JT):
                a, c = bn_coeffs(st2g, jh, g2s, be2s)
                nc.scalar.activation(recv[jh][:], recv[jh][:],
                                     Act.Relu, bias=c[:], scale=a[:])

            # ---------------- readout: z1 partial [64, 128] -----------------
            z1p_t = ps.tile([128, 512], f32, tag="big", bufs=4, name="z1p_t")
            z1p = z1p_t[:64, :128]
            rvs = [recv[jh][:].rearrange("f (src s p) -> f src s p",
                                         src=NCORES, s=BLOC, p=TSL)
                   for jh in range(JT)]
            for jh in range(JT):
                for pt in range(TSL):
                    tau = jh * TSL + pt
                    nc.tensor.matmul(
                        z1p, rvs[jh][:, :, :, pt],
                        Wm1s[:, tau * 128:(tau + 1) * 128],
                        start=(tau == 0), stop=(tau == NTAU - 1))
            z1s = sp.tile([64, 128], f32, tag="z1s")
            nc.vector.tensor_copy(z1s[:], z1p)
            nc.sync.dma_start(out=z1_in[:], in_=z1s[:])
            cc("AllReduce", Alu.add, z1_in, z1_out)
            z1g = sp.tile([64, 128], f32, tag="z1g")
            nc.sync.dma_start(out=z1g[:], in_=z1_out[:])

            z1t_t = ps.tile([128, 512], f32, tag="big", bufs=4, name="z1t_t")
            z1t_ps = z1t_t[:, :64]
            nc.tensor.transpose(z1t_ps, z1g[:], idents[:64, :64])
            z1t = sp.tile([128, 64], f32, tag="z1t")
            nc.vector.tensor_copy(z1t[:], z1t_ps)

            # ---------------- head BN + relu ----------------
            def head_bn(zt, parts, gs, bes):
                stm = sp.tile([parts, 1], f32, tag="hstm")
                nc.vector.reduce_sum(stm[:], zt[:], AX.X)
                mean = sp.tile([parts, 1], f32, tag="hmean")
                nc.vector.tensor_scalar_mul(mean[:], stm[:], 1.0 / CNT2)
                sqs2 = sp.tile([parts, 64], f32, tag="hsq")
                sts = sp.tile([parts, 1], f32, tag="hsts")
                nc.scalar.activation(sqs2[:], zt[:], Act.Square,
                                     accum_out=sts[:])
                var = sp.tile([parts, 1], f32, tag="hvar")
                nc.vector.tensor_scalar_mul(var[:], sts[:], 1.0 / CNT2)
                msq = sp.tile([parts, 1], f32, tag="hmsq")
                nc.vector.tensor_mul(msq[:], mean[:], mean[:])
                nc.vector.tensor_sub(var[:], var[:], msq[:])
                sd = sp.tile([parts, 1], f32, tag="hsd")
                nc.scalar.activation(sd[:], var[:], Act.Sqrt,
                                     bias=epsb[:parts, :])
                rsd = sp.tile([parts, 1], f32, tag="hrsd")
                nc.vector.reciprocal(rsd[:], sd[:])
                a = sp.tile([parts, 1], f32, tag="ha")
                nc.vector.tensor_mul(a[:], gs[:], rsd[:])
                c = sp.tile([parts, 1], f32, tag="hc")
                nc.vector.tensor_mul(c[:], mean[:], a[:])
                nc.vector.tensor_sub(c[:], bes[:], c[:])
                nc.scalar.activation(zt[:], zt[:], Act.Relu, bias=c[:],
                                     scale=a[:])

            head_bn(z1t, 128, gm1s, bem1s)

            z2_t = ps.tile([128, 512], f32, tag="big", bufs=4, name="z2_t")
            z2_ps = z2_t[:64, :64]
            nc.tensor.matmul(z2_ps, Wm2s[:], z1t[:], start=True, stop=True)
            z2t = sp.tile([64, 64], f32, tag="z2t")
            nc.vector.tensor_copy(z2t[:], z2_ps)
            head_bn(z2t, 64, gm2s, bem2s)

            z3_t = ps.tile([128, 512], f32, tag="big", bufs=4, name="z3_t")
            z3_ps = z3_t[:2, :64]
            nc.tensor.matmul(z3_ps, Wm3s[:], z2t[:], start=True, stop=True)
            z3 = sp.tile([2, 64], f32, tag="z3")
            nc.vector.tensor_scalar_add(z3[:], z3_ps, bm3s[:])
            nc.sync.dma_start(out=out_ext[:], in_=z3[:])

    nc.finalize()
    return nc


_CACHE = {}


def prepare_in_maps(inputs):
    import ml_dtypes
    bf = ml_dtypes.bfloat16

    x = np.asarray(inputs["x"], np.float32)
    mask = np.asarray(inputs["edge_prior_mask"], np.float32)
    Wm1 = np.asarray(inputs["Wm1"], np.float32)

    mA = 0.5 * mask * (1.0 - np.eye(P, dtype=np.float32))
    mB = mA + np.eye(P, dtype=np.float32)

    def c2(v, parts):  # [2*parts] -> [parts, 2] column-per-tile packing
        return np.ascontiguousarray(
            np.asarray(v, np.float32).reshape(2, parts).T)

    common = {
        "W1": np.asarray(inputs["W1"], bf),
        "b1r": np.asarray(inputs["b1"], bf).reshape(1, H),
        "g1p": c2(inputs["g1"], 128), "be1p": c2(inputs["be1"], 128),
        "W2": np.asarray(inputs["W2"], bf),
        "b2r": np.asarray(inputs["b2"], bf).reshape(1, H),
        "g2p": c2(inputs["g2"], 128), "be2p": c2(inputs["be2"], 128),
        "mAT": np.ascontiguousarray(np.tile(mA.T, (1, 4))),
        "mBT": np.ascontiguousarray(np.tile(mB.T, (1, 4))),
        "gm1": np.asarray(inputs["gm1"], np.float32).reshape(128, 1),
        "bem1": np.asarray(inputs["bem1"], np.float32).reshape(128, 1),
        "Wm2": np.asarray(inputs["Wm2"], np.float32),
        "gm2": np.asarray(inputs["gm2"], np.float32).reshape(64, 1),
        "bem2": np.asarray(inputs["bem2"], np.float32).reshape(64, 1),
        "Wm3": np.asarray(inputs["Wm3"], np.float32),
        "bm3": np.asarray(inputs["bm3"], np.float32).reshape(2, 1),
        "ones_row16": np.ones((1, 128), bf),
        "ones_col": np.ones((128, 1), np.float32),
        "ones_col16": np.ones((128, 1), bf),
        "ident": np.eye(128, dtype=np.float32),
    }
    in_maps = []
    for c in range(NCORES):
        xc = x[c * BLOC:(c + 1) * BLOC].reshape(T, D)
        m = dict(common)
        m["xT"] = np.ascontiguousarray(xc.T).astype(bf)
        # Wm1 rows for core c: (c*64+pt)*256 + jh*128 + f  ->  [f,(jh,pt,o)]
        Wc = Wm1[c * FSL:(c + 1) * FSL, :].reshape(TSL, JT, 128, 128)
        m["Wm1s"] = np.ascontiguousarray(
            Wc.transpose(2, 1, 0, 3).reshape(128, NTAU * 128)).astype(bf)
        in_maps.append(m)
    return in_maps


def kernel(**inputs):
    import concourse.bass_utils as bass_utils

    in_maps = prepare_in_maps(inputs)
    if "nc" not in _CACHE:
        _CACHE["nc"] = build_bass()
    res = bass_utils.run_bass_kernel_spmd(
        _CACHE["nc"], in_maps, core_ids=list(range(NCORES)))
    _CACHE["last"] = res
    out = res.results[0]["out"]  # [2, 64]
    return np.ascontiguousarray(np.asarray(out).T)
